# revision 1
# baseline (speedup 1.0000x reference)
"""Trainium2 Bass/Tile kernel v3: transposed-gate 2-layer biLSTM.

Layout: gates live in PARTITIONS (8 chunks of 128), batch in the free
dim.  PSUM trio tile = [128, (gc=8, sub=128)] where sub = r*32+b for
slot r (only sub 0:96 used).  Matmuls keep weights stationary
(LDWEIGHTS [128,128] bf16 -> FWL) and stream x / hT as rhs (N=96/32).
h/2 emerges feature-major from one fused stt -> no transposes at all;
layer-0 h is written straight into h0T, which feeds both the
recurrent matmul rhs and layer-1's GEMM rhs.

Conventions (host-compensated): c/2 tracked on device; g via
sigma(2x); h stored as h/2; output bf16 [T,2,2,128,32], host
reassembles and multiplies by 2.
"""

import os
import sys

import numpy as np

for _p in ("/opt/trn_rl_repo", "/root/.axon_site/_ro/trn_rl_repo"):
    if os.path.isdir(_p) and _p not in sys.path:
        sys.path.insert(0, _p)

from contextlib import ExitStack

import concourse.bass as bass  # noqa: F401
import concourse.mybir as mybir
import concourse.tile as tile
from concourse import bacc, bass_utils

AF = mybir.ActivationFunctionType
ALU = mybir.AluOpType
F32 = mybir.dt.float32
F32R = mybir.dt.float32r
BF16 = mybir.dt.bfloat16

T, B, CIN, H = 160, 256, 512, 256
G = 4 * H
NCORES = 8
BC = B // NCORES  # 32

_PERM = np.concatenate(
    [np.arange(0, 512), np.arange(768, 1024), np.arange(512, 768)]
)

_CACHE = {}


def _build():
    nc = bacc.Bacc("TRN2", target_bir_lowering=False, debug=False)

    xT_d = nc.dram_tensor("xT", [CIN, T * BC], BF16, kind="ExternalInput").ap()
    wih_d = [
        [
            nc.dram_tensor(f"wih{l}{d}", [128, 4 * G], BF16, kind="ExternalInput").ap()
            for d in (0, 1)
        ]
        for l in (0, 1)
    ]
    whh_d = [
        [
            nc.dram_tensor(f"whh{l}{d}", [128, 2 * G], BF16, kind="ExternalInput").ap()
            for d in (0, 1)
        ]
        for l in (0, 1)
    ]
    bias_d = [
        [
            nc.dram_tensor(f"bias{l}{d}", [1, G], F32R, kind="ExternalInput").ap()
            for d in (0, 1)
        ]
        for l in (0, 1)
    ]
    ones_d = nc.dram_tensor("ones", [1, 128], F32R, kind="ExternalInput").ap()
    zeros_d = nc.dram_tensor("zeros", [128, 128], BF16, kind="ExternalInput").ap()
    out_d = nc.dram_tensor(
        "out", [T, 2, 128, 64], BF16, kind="ExternalOutput"
    ).ap()

    with tile.TileContext(nc) as tc, ExitStack() as ctx:
        sb = ctx.enter_context(tc.tile_pool(name="sb", bufs=2))
        const = ctx.enter_context(tc.tile_pool(name="const", bufs=1))
        big = ctx.enter_context(tc.tile_pool(name="big", bufs=1))
        ps = ctx.enter_context(tc.tile_pool(name="ps", bufs=2, space="PSUM"))

        ones_sb = const.tile([1, 128], F32R)
        nc.sync.dma_start(ones_sb[:], ones_d[:])
        zeros_sb = const.tile([128, 128], BF16)
        nc.sync.dma_start(zeros_sb[:], zeros_d[:])

        # h0 storage, feature-major: [128, (k=2, dd=2, t=TP, b=32)] bf16
        TP = T + 4
        h0T = big.tile([128, 2 * 2 * TP * BC], BF16)
        h0T_r = h0T[:].rearrange("p (k dd t b) -> p k dd t b", k=2, dd=2, t=TP)
        for k in (0, 1):
            for dd in (0, 1):
                nc.sync.dma_start(
                    h0T_r[:, k, dd, T:TP, :].rearrange("p t b -> p (t b)"),
                    zeros_d[:, 0:128].rearrange("p (t b) -> p t b", t=4),
                )

        for l in (0, 1):
            wih_sb = [
                sb.tile([128, 4 * G], BF16, tag=f"wih{d}", bufs=1, name=f"wih{l}{d}s")
                for d in (0, 1)
            ]
            whh_sb = [
                sb.tile([128, 2 * G], BF16, tag=f"whh{d}", bufs=1, name=f"whh{l}{d}s")
                for d in (0, 1)
            ]
            bias_sb = [
                sb.tile([1, G], F32R, tag=f"bias{d}", bufs=1, name=f"bias{l}{d}s")
                for d in (0, 1)
            ]
            for d in (0, 1):
                nc.sync.dma_start(wih_sb[d][:], wih_d[l][d][:])
                nc.sync.dma_start(whh_sb[d][:], whh_d[l][d][:])
                nc.sync.dma_start(bias_sb[d][:], bias_d[l][d][:])
            # weight chunk views: wih [128, (gc, ki, 128)], whh [128, (gc, k, 128)]
            wih_r = [wih_sb[d][:].rearrange("p (gc ki g) -> p gc ki g", gc=8, ki=4)
                     for d in (0, 1)]
            whh_r = [whh_sb[d][:].rearrange("p (gc k g) -> p gc k g", gc=8, k=2)
                     for d in (0, 1)]

            def trio_len(q):
                return 3 if q < 53 else 1

            def t_of_slot(d, q, r):
                if d == 0:
                    return 3 * q + r
                return T - 3 * q - trio_len(q) + r

            def slot_of_step(d, s):
                q = min(s // 3, 53)
                r_in = s - 3 * q
                return r_in if d == 0 else trio_len(q) - 1 - r_in

            qt = {}
            stat = {}

            def emit_stat_dma(d, q, ki):
                w = 32 * trio_len(q)
                st = sb.tile([128, 96], BF16, tag="stat", bufs=16)
                c0 = t_of_slot(d, q, 0) * BC
                nc.sync.dma_start(
                    st[:, 0:w], xT_d[ki * 128 : (ki + 1) * 128, c0 : c0 + w]
                )
                stat[(d, q, ki)] = st

            def get_qt(d, q):
                if (d, q) not in qt:
                    t_ = ps.tile([128, G], F32, tag=f"qt{d}", name=f"qt{l}{d}_{q}")
                    qt[(d, q)] = (
                        t_,
                        t_[:].rearrange("p (gc sub) -> p gc sub", gc=8),
                    )
                return qt[(d, q)]

            def emit_gemm_chunk(d, q, ki):
                _, tr = get_qt(d, q)
                w = 32 * trio_len(q)
                if l == 0:
                    rhs = stat.pop((d, q, ki))[:, 0:w]
                else:
                    dsrc, k = ki // 2, ki % 2
                    t0 = t_of_slot(d, q, 0)
                    rhs = h0T_r[:, k, dsrc, t0 : t0 + trio_len(q), :].rearrange(
                        "p t b -> p (t b)"
                    )
                for gc in range(8):
                    # start=True clears has_written for the WHOLE bank, so
                    # only the first chunk touching each bank may set it
                    # (gc 0 -> bank A, gc 4 -> bank B); later chunks' first
                    # write lands on cleared bits and overwrites correctly.
                    nc.tensor.matmul(
                        tr[:, gc, 0:w],
                        wih_r[d][:, gc, ki, :],
                        rhs,
                        start=(ki == 0 and gc % 4 == 0),
                        stop=False,
                    )

            def emit_bias(d, q):
                t_, tr = get_qt(d, q)
                ln = trio_len(q)
                for gc in range(8):
                    nc.tensor.matmul(
                        tr[:, gc, 0 : 32 * ln],
                        bias_sb[d][:, gc * 128 : (gc + 1) * 128],
                        ones_sb[:, 0 : 32 * ln],
                        start=False,
                        stop=False,
                    )
                if q == 0:
                    # close the s=0 slot with zero-contribution matmuls
                    r0 = slot_of_step(d, 0)
                    for gc in range(8):
                        nc.tensor.matmul(
                            tr[:, gc, 32 * r0 : 32 * r0 + 32],
                            whh_r[d][:, gc, 0, :],
                            zeros_sb[:, 0:32],
                            start=False,
                            stop=True,
                        )

            # prefill trio 0 (+ stat DMAs for trios 0/1)
            if l == 0:
                for q in (0, 1):
                    for d in (0, 1):
                        for ki in range(4):
                            emit_stat_dma(d, q, ki)
            for d in (0, 1):
                for ki in range(4):
                    emit_gemm_chunk(d, 0, ki)
                emit_bias(d, 0)

            c_prev = [None, None]
            hT_prev = [None, None]
            for s in range(T):
                q = min(s // 3, 53)
                r = s - 3 * q

                # ---- GEMM prefetch for trio q+1 ----------------------------
                if q + 1 <= 53:
                    for d in (0, 1):
                        emit_gemm_chunk(d, q + 1, r)
                        if l == 0 and q + 2 <= 53:
                            emit_stat_dma(d, q + 2, r)
                    if r == 2:
                        for d in (0, 1):
                            if l == 0 and q + 2 <= 53:
                                emit_stat_dma(d, q + 2, 3)
                            emit_gemm_chunk(d, q + 1, 3)
                            emit_bias(d, q + 1)

                # ---- recurrent matmuls ------------------------------------
                for d in (0, 1):
                    if s == 0:
                        continue
                    rr = slot_of_step(d, s)
                    _, tr = get_qt(d, q)
                    hv = hT_prev[d][:].rearrange("p (k b) -> p k b", k=2)
                    rhs2 = [hv[:, k, :] for k in (0, 1)]
                    for gc in range(8):
                        for k in (0, 1):
                            nc.tensor.matmul(
                                tr[:, gc, 32 * rr : 32 * rr + 32],
                                whh_r[d][:, gc, k, :],
                                rhs2[k],
                                start=False,
                                stop=(k == 1),
                            )

                # ---- elementwise chains -----------------------------------
                for d in (0, 1):
                    rr = slot_of_step(d, s)
                    _, tr = get_qt(d, q)
                    sg = sb.tile([128, 256], F32, tag=f"sg{d}", bufs=2)
                    nc.scalar.activation(
                        sg[:].rearrange("p (gc b) -> p gc b", gc=8),
                        tr[:, :, 32 * rr : 32 * rr + 32],
                        AF.Sigmoid,
                    )
                    # i=sg[:,0:64] f=..64:128 o=..128:192 g'=..192:256
                    ig = sb.tile([128, 64], F32, tag=f"ig{d}", bufs=2)
                    nc.vector.scalar_tensor_tensor(
                        ig[:], sg[:, 192:256], 0.5, sg[:, 0:64],
                        ALU.subtract, ALU.mult,
                    )
                    c_new = sb.tile([128, 64], F32, tag=f"c{d}", bufs=2)
                    if s == 0:
                        nc.vector.tensor_copy(c_new[:], ig[:])
                    else:
                        fc = sb.tile([128, 64], F32, tag=f"fc{d}", bufs=2)
                        nc.vector.scalar_tensor_tensor(
                            fc[:], c_prev[d][:], 0.0, sg[:, 64:128],
                            ALU.add, ALU.mult,
                        )
                        nc.vector.scalar_tensor_tensor(
                            c_new[:], fc[:], 0.0, ig[:], ALU.add, ALU.add
                        )
                    c_prev[d] = c_new
                    sc = sb.tile([128, 64], F32, tag=f"sc{d}", bufs=2)
                    nc.scalar.activation(sc[:], c_new[:], AF.Sigmoid, scale=4.0)

                    t_nat = s if d == 0 else T - 1 - s
                    hT_new = sb.tile([128, 64], BF16, tag=f"hT{d}", bufs=3)
                    nc.vector.scalar_tensor_tensor(
                        hT_new[:], sc[:], 0.5, sg[:, 128:192],
                        ALU.subtract, ALU.mult,
                    )
                    hT_prev[d] = hT_new
                    if l == 0:
                        nc.vector.tensor_copy(
                            h0T_r[:, :, d, t_nat, :],
                            hT_new[:].rearrange("p (k b) -> p k b", k=2),
                        )
                    else:
                        nc.sync.dma_start(out_d[t_nat, d], hT_new[:])

    nc.compile()
    return nc


def _prep_inputs(inputs):
    x = np.asarray(inputs["x"], dtype=np.float32)
    import ml_dtypes

    bf = ml_dtypes.bfloat16
    common = {}
    sv = np.ones(G, np.float32)
    sv[768:] = 2.0
    for l in (0, 1):
        for d, sfx in enumerate(("", "_reverse")):
            Wih = np.asarray(inputs[f"weight_ih_l{l}{sfx}"], dtype=np.float32)
            Whh = np.asarray(inputs[f"weight_hh_l{l}{sfx}"], dtype=np.float32)
            bsum = (
                np.asarray(inputs[f"bias_ih_l{l}{sfx}"], dtype=np.float32)
                + np.asarray(inputs[f"bias_hh_l{l}{sfx}"], dtype=np.float32)
            )
            wihT = np.ascontiguousarray(Wih.T[:, _PERM])
            whhT = np.ascontiguousarray(Whh.T[:, _PERM])
            bias = bsum[_PERM]
            wih_scale = sv * (1.0 if l == 0 else 2.0)
            whh_scale = sv * 2.0
            wihT = wihT * wih_scale[None, :]
            whhT = whhT * whh_scale[None, :]
            bias = bias * sv
            # wih: [cin,1024] -> [4ki,128p,8gc,128g] -> [128, (gc,ki,g)]
            common[f"wih{l}{d}"] = np.ascontiguousarray(
                wihT.reshape(4, 128, 8, 128).transpose(1, 2, 0, 3).reshape(128, 4 * G),
                dtype=bf,
            )
            # whh: [256,1024] -> [2k,128p,8gc,128g] -> [128, (gc,k,g)]
            common[f"whh{l}{d}"] = np.ascontiguousarray(
                whhT.reshape(2, 128, 8, 128).transpose(1, 2, 0, 3).reshape(128, 2 * G),
                dtype=bf,
            )
            common[f"bias{l}{d}"] = np.ascontiguousarray(
                bias[None, :], dtype=np.float32
            )
    common["ones"] = np.ones((1, 128), dtype=np.float32)
    common["zeros"] = np.zeros((128, 128), dtype=bf)

    in_maps = []
    for c in range(NCORES):
        xs = x[:, c * BC : (c + 1) * BC, :]
        m = dict(common)
        m["xT"] = np.ascontiguousarray(
            xs.transpose(2, 0, 1).reshape(CIN, T * BC), dtype=bf
        )
        in_maps.append(m)
    return in_maps


def _get_program():
    if "prog" not in _CACHE:
        _CACHE["prog"] = _build()
    return _CACHE["prog"]


def kernel(**inputs):
    nc = _get_program()
    in_maps = _prep_inputs(inputs)
    res = bass_utils.run_bass_kernel_spmd(nc, in_maps, core_ids=list(range(NCORES)))
    out = np.empty((T, B, 2 * H), np.float32)
    for c in range(NCORES):
        o = res.results[c]["out"].astype(np.float32)  # [T,2,128,(k,b)]
        o = o.reshape(T, 2, 128, 2, 32)  # t, d, p, k, b
        # feature index = d*256 + k*128 + p ; batch from b
        o = o.transpose(0, 4, 1, 3, 2).reshape(T, BC, 512)
        out[:, c * BC : (c + 1) * BC, :] = o
    out *= 2.0
    return out



# revision 5
# speedup vs baseline: 1.0750x; 1.0750x over previous
"""Trainium2 Bass/Tile kernel v3: transposed-gate 2-layer biLSTM.

Layout: gates live in PARTITIONS (8 chunks of 128), batch in the free
dim.  PSUM trio tile = [128, (gc=8, sub=128)] where sub = r*32+b for
slot r (only sub 0:96 used).  Matmuls keep weights stationary
(LDWEIGHTS [128,128] bf16 -> FWL) and stream x / hT as rhs (N=96/32).
h/2 emerges feature-major from one fused stt -> no transposes at all;
layer-0 h is written straight into h0T, which feeds both the
recurrent matmul rhs and layer-1's GEMM rhs.

Conventions (host-compensated): c/2 tracked on device; g via
sigma(2x); h stored as h/2; output bf16 [T,2,2,128,32], host
reassembles and multiplies by 2.
"""

import os
import sys

import numpy as np

for _p in ("/opt/trn_rl_repo", "/root/.axon_site/_ro/trn_rl_repo"):
    if os.path.isdir(_p) and _p not in sys.path:
        sys.path.insert(0, _p)

from contextlib import ExitStack

import concourse.bass as bass  # noqa: F401
import concourse.mybir as mybir
import concourse.tile as tile
from concourse import bacc, bass_utils

AF = mybir.ActivationFunctionType
ALU = mybir.AluOpType
F32 = mybir.dt.float32
F32R = mybir.dt.float32r
BF16 = mybir.dt.bfloat16
FP8 = mybir.dt.float8e4

T, B, CIN, H = 160, 256, 512, 256
G = 4 * H
NCORES = 8
BC = B // NCORES  # 32

_PERM = np.concatenate(
    [np.arange(0, 512), np.arange(768, 1024), np.arange(512, 768)]
)

_CACHE = {}


def _build():
    nc = bacc.Bacc("TRN2", target_bir_lowering=False, debug=False)

    xT_d = nc.dram_tensor("xT", [CIN, T * BC], BF16, kind="ExternalInput").ap()
    wih_d = [
        [
            nc.dram_tensor(f"wih{l}{d}", [128, 4 * G], BF16, kind="ExternalInput").ap()
            for d in (0, 1)
        ]
        for l in (0, 1)
    ]
    whh_d = [
        [
            nc.dram_tensor(f"whh{l}{d}", [128, 2 * G], FP8, kind="ExternalInput").ap()
            for d in (0, 1)
        ]
        for l in (0, 1)
    ]
    bias_d = [
        [
            nc.dram_tensor(f"bias{l}{d}", [1, G], F32R, kind="ExternalInput").ap()
            for d in (0, 1)
        ]
        for l in (0, 1)
    ]
    ones_d = nc.dram_tensor("ones", [1, 128], F32R, kind="ExternalInput").ap()
    zeros_d = nc.dram_tensor("zeros", [128, 128], BF16, kind="ExternalInput").ap()
    out_d = nc.dram_tensor(
        "out", [T, 2, 128, 64], BF16, kind="ExternalOutput"
    ).ap()

    with tile.TileContext(nc) as tc, ExitStack() as ctx:
        sb = ctx.enter_context(tc.tile_pool(name="sb", bufs=2))
        const = ctx.enter_context(tc.tile_pool(name="const", bufs=1))
        big = ctx.enter_context(tc.tile_pool(name="big", bufs=1))
        ps = ctx.enter_context(tc.tile_pool(name="ps", bufs=2, space="PSUM"))

        ones_sb = const.tile([1, 128], F32R)
        nc.sync.dma_start(ones_sb[:], ones_d[:])
        zeros_sb = const.tile([128, 128], BF16)
        nc.sync.dma_start(zeros_sb[:], zeros_d[:])

        # h0 storage, feature-major: [128, (k=2, dd=2, t=TP, b=32)] bf16
        TP = T + 4
        h0T = big.tile([128, 2 * 2 * TP * BC], BF16)
        h0T_r = h0T[:].rearrange("p (k dd t b) -> p k dd t b", k=2, dd=2, t=TP)
        for k in (0, 1):
            for dd in (0, 1):
                nc.sync.dma_start(
                    h0T_r[:, k, dd, T:TP, :].rearrange("p t b -> p (t b)"),
                    zeros_d[:, 0:128].rearrange("p (t b) -> p t b", t=4),
                )

        for l in (0, 1):
            wih_sb = [
                sb.tile([128, 4 * G], BF16, tag=f"wih{d}", bufs=1, name=f"wih{l}{d}s")
                for d in (0, 1)
            ]
            whh_sb = [
                sb.tile([128, 2 * G], FP8, tag=f"whh{d}", bufs=1, name=f"whh{l}{d}s")
                for d in (0, 1)
            ]
            bias_sb = [
                sb.tile([1, G], F32R, tag=f"bias{d}", bufs=1, name=f"bias{l}{d}s")
                for d in (0, 1)
            ]
            for d in (0, 1):
                nc.sync.dma_start(wih_sb[d][:], wih_d[l][d][:])
                nc.sync.dma_start(whh_sb[d][:], whh_d[l][d][:])
                nc.sync.dma_start(bias_sb[d][:], bias_d[l][d][:])
            # weight chunk views: wih [128, (gc, ki, 128)], whh [128, (gc, k, 128)]
            wih_r = [wih_sb[d][:].rearrange("p (gc ki g) -> p gc ki g", gc=8, ki=4)
                     for d in (0, 1)]
            whh_r = [whh_sb[d][:].rearrange("p (gc k g) -> p gc k g", gc=8, k=2)
                     for d in (0, 1)]

            def trio_len(q):
                return 3 if q < 53 else 1

            def t_of_slot(d, q, r):
                if d == 0:
                    return 3 * q + r
                return T - 3 * q - trio_len(q) + r

            def slot_of_step(d, s):
                q = min(s // 3, 53)
                r_in = s - 3 * q
                return r_in if d == 0 else trio_len(q) - 1 - r_in

            qt = {}
            stat = {}

            def emit_stat_dma(d, q, ki):
                w = 32 * trio_len(q)
                st = sb.tile([128, 96], BF16, tag="stat", bufs=16)
                c0 = t_of_slot(d, q, 0) * BC
                nc.sync.dma_start(
                    st[:, 0:w], xT_d[ki * 128 : (ki + 1) * 128, c0 : c0 + w]
                )
                stat[(d, q, ki)] = st

            def get_qt(d, q):
                if (d, q) not in qt:
                    t_ = ps.tile([128, G], F32, tag=f"qt{d}", name=f"qt{l}{d}_{q}")
                    qt[(d, q)] = (
                        t_,
                        t_[:].rearrange("p (gc sub) -> p gc sub", gc=8),
                    )
                return qt[(d, q)]

            def emit_gemm_chunk(d, q, ki):
                _, tr = get_qt(d, q)
                w = 32 * trio_len(q)
                if l == 0:
                    rhs = stat.pop((d, q, ki))[:, 0:w]
                else:
                    dsrc, k = ki // 2, ki % 2
                    t0 = t_of_slot(d, q, 0)
                    rhs = h0T_r[:, k, dsrc, t0 : t0 + trio_len(q), :].rearrange(
                        "p t b -> p (t b)"
                    )
                for gc in range(8):
                    # start=True clears has_written for the WHOLE bank, so
                    # only the first chunk touching each bank may set it
                    # (gc 0 -> bank A, gc 4 -> bank B); later chunks' first
                    # write lands on cleared bits and overwrites correctly.
                    nc.tensor.matmul(
                        tr[:, gc, 0:w],
                        wih_r[d][:, gc, ki, :],
                        rhs,
                        start=(ki == 0 and gc % 4 == 0),
                        stop=False,
                    )

            def emit_bias(d, q):
                t_, tr = get_qt(d, q)
                ln = trio_len(q)
                for gc in range(8):
                    nc.tensor.matmul(
                        tr[:, gc, 0 : 32 * ln],
                        bias_sb[d][:, gc * 128 : (gc + 1) * 128],
                        ones_sb[:, 0 : 32 * ln],
                        start=False,
                        stop=False,
                    )
                if q == 0:
                    # close the s=0 slot with zero-contribution matmuls
                    r0 = slot_of_step(d, 0)
                    for gc in range(8):
                        nc.tensor.matmul(
                            tr[:, gc, 32 * r0 : 32 * r0 + 32],
                            whh_r[d][:, gc, 0, :],
                            zeros_sb[:, 0:32],
                            start=False,
                            stop=True,
                        )

            # prefill trio 0 (+ stat DMAs for trios 0/1)
            if l == 0:
                for q in (0, 1):
                    for d in (0, 1):
                        for ki in range(4):
                            emit_stat_dma(d, q, ki)
            for d in (0, 1):
                for ki in range(4):
                    emit_gemm_chunk(d, 0, ki)
                emit_bias(d, 0)

            c_prev = [None, None]
            hT_prev = [None, None]
            for s in range(T):
                q = min(s // 3, 53)
                r = s - 3 * q

                # ---- GEMM prefetch for trio q+1 ----------------------------
                if q + 1 <= 53:
                    for d in (0, 1):
                        emit_gemm_chunk(d, q + 1, r)
                        if l == 0 and q + 2 <= 53:
                            emit_stat_dma(d, q + 2, r)
                    if r == 2:
                        for d in (0, 1):
                            if l == 0 and q + 2 <= 53:
                                emit_stat_dma(d, q + 2, 3)
                            emit_gemm_chunk(d, q + 1, 3)
                            emit_bias(d, q + 1)

                # ---- recurrent matmuls ------------------------------------
                for d in (0, 1):
                    if s == 0:
                        continue
                    rr = slot_of_step(d, s)
                    _, tr = get_qt(d, q)
                    hv = hT_prev[d][:].rearrange("p (k b) -> p k b", k=2)
                    rhs2 = [hv[:, k, :] for k in (0, 1)]
                    for gc in range(8):
                        for k in (0, 1):
                            nc.tensor.matmul(
                                tr[:, gc, 32 * rr : 32 * rr + 32],
                                whh_r[d][:, gc, k, :],
                                rhs2[k],
                                start=False,
                                stop=(k == 1),
                            )

                # ---- elementwise chains -----------------------------------
                for d in (0, 1):
                    rr = slot_of_step(d, s)
                    _, tr = get_qt(d, q)
                    sg = sb.tile([128, 256], F32, tag=f"sg{d}", bufs=2)
                    nc.scalar.activation(
                        sg[:].rearrange("p (gc b) -> p gc b", gc=8),
                        tr[:, :, 32 * rr : 32 * rr + 32],
                        AF.Sigmoid,
                    )
                    # i=sg[:,0:64] f=..64:128 o=..128:192 g'=..192:256
                    ig = sb.tile([128, 64], F32, tag=f"ig{d}", bufs=2)
                    nc.vector.scalar_tensor_tensor(
                        ig[:], sg[:, 192:256], 0.5, sg[:, 0:64],
                        ALU.subtract, ALU.mult,
                    )
                    c_new = sb.tile([128, 64], F32, tag=f"c{d}", bufs=2)
                    if s == 0:
                        nc.vector.tensor_copy(c_new[:], ig[:])
                    else:
                        fc = sb.tile([128, 64], F32, tag=f"fc{d}", bufs=2)
                        nc.vector.scalar_tensor_tensor(
                            fc[:], c_prev[d][:], 0.0, sg[:, 64:128],
                            ALU.add, ALU.mult,
                        )
                        nc.vector.scalar_tensor_tensor(
                            c_new[:], fc[:], 0.0, ig[:], ALU.add, ALU.add
                        )
                    c_prev[d] = c_new
                    sc = sb.tile([128, 64], F32, tag=f"sc{d}", bufs=2)
                    nc.scalar.activation(sc[:], c_new[:], AF.Sigmoid, scale=4.0)

                    t_nat = s if d == 0 else T - 1 - s
                    hT_new = sb.tile([128, 64], BF16, tag=f"hT{d}", bufs=3)
                    nc.vector.scalar_tensor_tensor(
                        hT_new[:], sc[:], 0.5, sg[:, 128:192],
                        ALU.subtract, ALU.mult,
                    )
                    hT_prev[d] = hT_new
                    if l == 0:
                        nc.vector.tensor_copy(
                            h0T_r[:, :, d, t_nat, :],
                            hT_new[:].rearrange("p (k b) -> p k b", k=2),
                        )
                    else:
                        nc.sync.dma_start(out_d[t_nat, d], hT_new[:])

    nc.compile()
    return nc


def _prep_inputs(inputs):
    x = np.asarray(inputs["x"], dtype=np.float32)
    import ml_dtypes

    bf = ml_dtypes.bfloat16
    common = {}
    sv = np.ones(G, np.float32)
    sv[768:] = 2.0
    for l in (0, 1):
        for d, sfx in enumerate(("", "_reverse")):
            Wih = np.asarray(inputs[f"weight_ih_l{l}{sfx}"], dtype=np.float32)
            Whh = np.asarray(inputs[f"weight_hh_l{l}{sfx}"], dtype=np.float32)
            bsum = (
                np.asarray(inputs[f"bias_ih_l{l}{sfx}"], dtype=np.float32)
                + np.asarray(inputs[f"bias_hh_l{l}{sfx}"], dtype=np.float32)
            )
            wihT = np.ascontiguousarray(Wih.T[:, _PERM])
            whhT = np.ascontiguousarray(Whh.T[:, _PERM])
            bias = bsum[_PERM]
            wih_scale = sv * (1.0 if l == 0 else 2.0)
            whh_scale = sv * 2.0
            wihT = wihT * wih_scale[None, :]
            whhT = whhT * whh_scale[None, :]
            bias = bias * sv
            # wih: [cin,1024] -> [4ki,128p,8gc,128g] -> [128, (gc,ki,g)]
            common[f"wih{l}{d}"] = np.ascontiguousarray(
                wihT.reshape(4, 128, 8, 128).transpose(1, 2, 0, 3).reshape(128, 4 * G),
                dtype=bf,
            )
            # whh: [256,1024] -> [2k,128p,8gc,128g] -> [128, (gc,k,g)]
            common[f"whh{l}{d}"] = np.ascontiguousarray(
                whhT.reshape(2, 128, 8, 128).transpose(1, 2, 0, 3).reshape(128, 2 * G),
                dtype=ml_dtypes.float8_e4m3fn,
            )
            common[f"bias{l}{d}"] = np.ascontiguousarray(
                bias[None, :], dtype=np.float32
            )
    common["ones"] = np.ones((1, 128), dtype=np.float32)
    common["zeros"] = np.zeros((128, 128), dtype=bf)

    in_maps = []
    for c in range(NCORES):
        xs = x[:, c * BC : (c + 1) * BC, :]
        m = dict(common)
        m["xT"] = np.ascontiguousarray(
            xs.transpose(2, 0, 1).reshape(CIN, T * BC), dtype=bf
        )
        in_maps.append(m)
    return in_maps


def _get_program():
    if "prog" not in _CACHE:
        _CACHE["prog"] = _build()
    return _CACHE["prog"]


def kernel(**inputs):
    nc = _get_program()
    in_maps = _prep_inputs(inputs)
    res = bass_utils.run_bass_kernel_spmd(nc, in_maps, core_ids=list(range(NCORES)))
    out = np.empty((T, B, 2 * H), np.float32)
    for c in range(NCORES):
        o = res.results[c]["out"].astype(np.float32)  # [T,2,128,(k,b)]
        o = o.reshape(T, 2, 128, 2, 32)  # t, d, p, k, b
        # feature index = d*256 + k*128 + p ; batch from b
        o = o.transpose(0, 4, 1, 3, 2).reshape(T, BC, 512)
        out[:, c * BC : (c + 1) * BC, :] = o
    out *= 2.0
    return out



# revision 8
# speedup vs baseline: 1.0949x; 1.0185x over previous
"""Trainium2 Bass/Tile kernel v6: bulk-GEMM + latency-optimized scan.

vs v5: xg is injected into the recurrent PSUM tile by identity-weight
matmuls (no DVE adds, no s==0 special case), the two direction chains
are de-merged (independent recurrences, phase-offset pipelines), fc
runs on GpSimd in parallel with ig on DVE, and PE work is ordered
(GEMM unit, injects, recs) so free work drains while rec waits on h.
"""

import os
import sys

import numpy as np

for _p in ("/opt/trn_rl_repo", "/root/.axon_site/_ro/trn_rl_repo"):
    if os.path.isdir(_p) and _p not in sys.path:
        sys.path.insert(0, _p)

from contextlib import ExitStack

import concourse.bass as bass  # noqa: F401
import concourse.mybir as mybir
import concourse.tile as tile
from concourse import bacc, bass_utils

AF = mybir.ActivationFunctionType
ALU = mybir.AluOpType
F32 = mybir.dt.float32
BF16 = mybir.dt.bfloat16

T, B, CIN, H = 160, 256, 512, 256
G = 4 * H  # 1024
NCORES = 8
BC = B // NCORES  # 32
TC = 16            # timesteps per GEMM chunk
NCH = T // TC      # 10
RING = 48          # xg ring slots (3 chunks)

_PERM = np.concatenate(
    [np.arange(0, 512), np.arange(768, 1024), np.arange(512, 768)]
)

_CACHE = {}
FC_ON_GPSIMD = os.environ.get("FC_GP", "0") == "1"


def _build():
    nc = bacc.Bacc("TRN2", target_bir_lowering=False, debug=False)

    xT_d = nc.dram_tensor("xT", [CIN, T * BC], BF16, kind="ExternalInput").ap()
    wih_d = [
        [
            nc.dram_tensor(f"wih{l}{d}", [128, 4 * G], BF16, kind="ExternalInput").ap()
            for d in (0, 1)
        ]
        for l in (0, 1)
    ]
    whh_d = [
        [
            nc.dram_tensor(f"whh{l}{d}", [128, 2 * G], BF16, kind="ExternalInput").ap()
            for d in (0, 1)
        ]
        for l in (0, 1)
    ]
    biasP_d = [
        [
            nc.dram_tensor(f"biasP{l}{d}", [128, 8], F32, kind="ExternalInput").ap()
            for d in (0, 1)
        ]
        for l in (0, 1)
    ]
    ident_d = nc.dram_tensor("ident", [128, 128], BF16, kind="ExternalInput").ap()
    out_d = nc.dram_tensor(
        "out", [T, 2, 128, 64], BF16, kind="ExternalOutput"
    ).ap()

    def t_nat(d, s):
        return s if d == 0 else T - 1 - s

    with tile.TileContext(nc) as tc, ExitStack() as ctx:
        sb = ctx.enter_context(tc.tile_pool(name="sb", bufs=2))
        const = ctx.enter_context(tc.tile_pool(name="const", bufs=1))
        big = ctx.enter_context(tc.tile_pool(name="big", bufs=1))
        ps = ctx.enter_context(tc.tile_pool(name="ps", bufs=2, space="PSUM"))

        ident_sb = const.tile([128, 128], BF16)
        nc.sync.dma_start(ident_sb[:], ident_d[:])

        # h0 storage, feature-major: [128, (k=2, dd=2, t=T, b=32)] bf16
        h0T = big.tile([128, 2 * 2 * T * BC], BF16)
        h0T_r = h0T[:].rearrange("p (k dd t b) -> p k dd t b", k=2, dd=2, t=T)
        # xg ring: [128, (d=2, gc=8, slot=RING, b=32)] bf16 -- gc-major so
        # the GEMM evict writes a contiguous [128,512] run per gate chunk
        ring = big.tile([128, 2 * RING * 8 * BC], BF16)
        ring_r = ring[:].rearrange("p (d gc sl b) -> p d gc sl b", d=2, gc=8, sl=RING)
        # layer-1 hT ring: [128, (sp=2, d=2, k=2, b=32)] bf16
        hr = big.tile([128, 2 * 2 * 2 * BC], BF16)
        hr_r = hr[:].rearrange("p (sp d k b) -> p sp d k b", sp=2, d=2, k=2)

        for l in (0, 1):
            wih_sb = [
                sb.tile([128, 4 * G], BF16, tag=f"wih{d}", bufs=1, name=f"wih{l}{d}s")
                for d in (0, 1)
            ]
            whh_sb = [
                sb.tile([128, 2 * G], BF16, tag=f"whh{d}", bufs=1, name=f"whh{l}{d}s")
                for d in (0, 1)
            ]
            biasP_sb = [
                sb.tile([128, 8], F32, tag=f"biasP{d}", bufs=1, name=f"biasP{l}{d}s")
                for d in (0, 1)
            ]
            for d in (0, 1):
                nc.sync.dma_start(wih_sb[d][:], wih_d[l][d][:])
                nc.sync.dma_start(whh_sb[d][:], whh_d[l][d][:])
                nc.sync.dma_start(biasP_sb[d][:], biasP_d[l][d][:])
            wih_r = [wih_sb[d][:].rearrange("p (gc ki g) -> p gc ki g", gc=8, ki=4)
                     for d in (0, 1)]
            whh_r = [whh_sb[d][:].rearrange("p (gc k g) -> p gc k g", gc=8, k=2)
                     for d in (0, 1)]

            # ---- input GEMM units ---------------------------------------
            xstat = {}
            pending_evicts = []

            def get_xstat(d, c_nat, ki):
                key = (d, c_nat, ki)
                if key not in xstat:
                    st = sb.tile([128, TC * BC], BF16, tag=f"x{d}", bufs=8)
                    c0 = c_nat * TC * BC
                    nc.sync.dma_start(
                        st[:], xT_d[ki * 128:(ki + 1) * 128, c0:c0 + TC * BC]
                    )
                    xstat[key] = st
                return xstat[key]

            def prefetch_x(d, tc_):
                # pull the x tiles for consumption-chunk tc_ well before
                # their matmuls hit the PE queue (avoids FIFO-head stalls)
                if l == 0 and tc_ < NCH:
                    c_nat = tc_ if d == 0 else NCH - 1 - tc_
                    for ki in range(4):
                        get_xstat(d, c_nat, ki)

            def emit_unit(d, u):
                tc_, gc = u // 8, u % 8
                c_nat = tc_ if d == 0 else NCH - 1 - tc_
                if gc == 0:
                    prefetch_x(d, tc_ + 1)
                pg = ps.tile([128, TC * BC], F32, tag=f"g{d}", bufs=2,
                             name=f"pg{l}{d}_{u}")
                for ki in range(4):
                    if l == 0:
                        rhs = get_xstat(d, c_nat, ki)[:]
                    else:
                        dsrc, k = ki // 2, ki % 2
                        rhs = h0T_r[:, k, dsrc, c_nat * TC:(c_nat + 1) * TC, :]\
                            .rearrange("p t b -> p (t b)")
                    nc.tensor.matmul(
                        pg[:], wih_r[d][:, gc, ki, :], rhs,
                        start=(ki == 0), stop=(ki == 3),
                    )
                slot0 = (c_nat * TC) % RING
                pending_evicts.append((pg, d, gc, slot0))

            def flush_evicts():
                # small priority defer: past this step's sc ops on the ACT
                # sequence, but well before the next step's sigmoid
                while pending_evicts:
                    pg, d, gc, slot0 = pending_evicts.pop(0)
                    with tc.high_priority(offset=-60):
                        nc.scalar.activation(
                            ring_r[:, d, gc, slot0:slot0 + TC, :]
                            .rearrange("p t b -> p (t b)"),
                            pg[:],
                            AF.Identity,
                            bias=biasP_sb[d][:, gc:gc + 1],
                        )

            # ---- scan ---------------------------------------------------
            prs = {}

            def emit_inject(d, s):
                # xg injection for step s: 8 identity matmuls, no h dep
                pr = ps.tile([128, 256], F32, tag=f"r{d}", bufs=2,
                             name=f"pr{l}{d}_{s}")
                pr_r = pr[:].rearrange("p (gc b) -> p gc b", gc=8)
                sl = t_nat(d, s) % RING
                for gc in range(8):
                    nc.tensor.matmul(
                        pr_r[:, gc, :], ident_sb[:],
                        ring_r[:, d, gc, sl, :],
                        start=(gc == 0), stop=(s == 0 and gc == 7),
                    )
                prs[(d, s)] = (pr, pr_r)

            def emit_rec(d, s):
                pr, pr_r = prs[(d, s)]
                if s == 0:
                    return
                if l == 0:
                    hv = [h0T_r[:, k, d, t_nat(d, s - 1), :] for k in (0, 1)]
                else:
                    hv = [hr_r[:, (s - 1) % 2, d, k, :] for k in (0, 1)]
                # k outer: all k=0 matmuls first so they can start as soon
                # as the k=0 half of h is written (hT is k-split)
                for k in (0, 1):
                    for gc in range(8):
                        nc.tensor.matmul(
                            pr_r[:, gc, :],
                            whh_r[d][:, gc, k, :],
                            hv[k],
                            start=False,
                            stop=(gc == 7 and k == 1),
                        )

            c_prev = [None, None]

            def emit_chain(d, s):
                pr, pr_r = prs.pop((d, s))
                sg = sb.tile([128, 256], F32, tag=f"sg{d}", bufs=2)
                nc.scalar.activation(sg[:], pr[:], AF.Sigmoid)
                sg_r = sg[:].rearrange("p (gc b) -> p gc b", gc=8)
                # gc chunks: i=0:2 f=2:4 o=4:6 g'=6:8
                ig = sb.tile([128, 64], F32, tag=f"ig{d}", bufs=2)
                nc.vector.scalar_tensor_tensor(
                    ig[:].rearrange("p (k b) -> p k b", k=2),
                    sg_r[:, 6:8, :], 0.5, sg_r[:, 0:2, :],
                    ALU.subtract, ALU.mult,
                )
                c_new = sb.tile([128, 64], F32, tag=f"c{d}", bufs=2)
                if s == 0:
                    nc.vector.tensor_copy(c_new[:], ig[:])
                else:
                    fc = sb.tile([128, 64], F32, tag=f"fc{d}", bufs=2)
                    eng = nc.gpsimd if FC_ON_GPSIMD else nc.vector
                    eng.scalar_tensor_tensor(
                        fc[:].rearrange("p (k b) -> p k b", k=2),
                        c_prev[d][:].rearrange("p (k b) -> p k b", k=2),
                        0.0, sg_r[:, 2:4, :],
                        ALU.add, ALU.mult,
                    )
                    nc.vector.scalar_tensor_tensor(
                        c_new[:], fc[:], 0.0, ig[:], ALU.add, ALU.add
                    )
                c_prev[d] = c_new
                sc = sb.tile([128, 64], F32, tag=f"sc{d}", bufs=2)
                nc.scalar.activation(sc[:], c_new[:], AF.Sigmoid, scale=4.0)
                sc_r = sc[:].rearrange("p (k b) -> p k b", k=2)
                # hT split by k-chunk: the k=0 half of h lands first so the
                # next step's k=0 recurrent matmuls can start immediately
                for k in (0, 1):
                    if l == 0:
                        hout = h0T_r[:, k, d, t_nat(d, s), :]
                    else:
                        hout = hr_r[:, s % 2, d, k, :]
                    nc.vector.scalar_tensor_tensor(
                        hout, sc_r[:, k, :], 0.5, sg_r[:, 4 + k, :],
                        ALU.subtract, ALU.mult,
                    )
                if l == 1:
                    nc.sync.dma_start(
                        out_d[t_nat(d, s), d],
                        hr_r[:, s % 2, d].rearrange("p k b -> p (k b)"),
                    )

            # prefill: x tiles first, then 10 GEMM units per dir (the
            # in-scan emission rate of 1 unit per 2 steps per dir keeps a
            # ~6-step slack from there on)
            for d in (0, 1):
                prefetch_x(d, 0)
            for u in range(10):
                for d in (0, 1):
                    emit_unit(d, u)
            flush_evicts()
            nunit = [10, 10]
            # prefill injects for step 0
            for d in (0, 1):
                emit_inject(d, 0)

            for s in range(T):
                # one GEMM unit per step (PE filler, no deps on scan)
                d_e = s % 2
                if nunit[d_e] < 80:
                    emit_unit(d_e, nunit[d_e])
                    nunit[d_e] += 1
                elif nunit[1 - d_e] < 80:
                    emit_unit(1 - d_e, nunit[1 - d_e])
                    nunit[1 - d_e] += 1
                # free PE work first: injects for s+1
                if s + 1 < T:
                    emit_inject(0, s + 1)
                    emit_inject(1, s + 1)
                # dependent PE work: recurrent matmuls
                emit_rec(0, s)
                emit_rec(1, s)
                # elementwise chains (d0 then d1 -> phase offset)
                emit_chain(0, s)
                emit_chain(1, s)
                # GEMM evict last: queues on ACT after the chain's sigmoids
                flush_evicts()

    nc.compile()
    return nc


def _prep_inputs(inputs):
    x = np.asarray(inputs["x"], dtype=np.float32)
    import ml_dtypes

    bf = ml_dtypes.bfloat16
    common = {}
    sv = np.ones(G, np.float32)
    sv[768:] = 2.0
    for l in (0, 1):
        for d, sfx in enumerate(("", "_reverse")):
            Wih = np.asarray(inputs[f"weight_ih_l{l}{sfx}"], dtype=np.float32)
            Whh = np.asarray(inputs[f"weight_hh_l{l}{sfx}"], dtype=np.float32)
            bsum = (
                np.asarray(inputs[f"bias_ih_l{l}{sfx}"], dtype=np.float32)
                + np.asarray(inputs[f"bias_hh_l{l}{sfx}"], dtype=np.float32)
            )
            wihT = np.ascontiguousarray(Wih.T[:, _PERM])
            whhT = np.ascontiguousarray(Whh.T[:, _PERM])
            bias = bsum[_PERM]
            wih_scale = sv * (1.0 if l == 0 else 2.0)
            whh_scale = sv * 2.0
            wihT = wihT * wih_scale[None, :]
            whhT = whhT * whh_scale[None, :]
            bias = bias * sv
            common[f"wih{l}{d}"] = np.ascontiguousarray(
                wihT.reshape(4, 128, 8, 128).transpose(1, 2, 0, 3).reshape(128, 4 * G),
                dtype=bf,
            )
            common[f"whh{l}{d}"] = np.ascontiguousarray(
                whhT.reshape(2, 128, 8, 128).transpose(1, 2, 0, 3).reshape(128, 2 * G),
                dtype=bf,
            )
            common[f"biasP{l}{d}"] = np.ascontiguousarray(
                bias.reshape(8, 128).T, dtype=np.float32
            )
    common["ident"] = np.eye(128, dtype=bf)

    in_maps = []
    for c in range(NCORES):
        xs = x[:, c * BC:(c + 1) * BC, :]
        m = dict(common)
        m["xT"] = np.ascontiguousarray(
            xs.transpose(2, 0, 1).reshape(CIN, T * BC), dtype=bf
        )
        in_maps.append(m)
    return in_maps


def _get_program():
    if "prog" not in _CACHE:
        _CACHE["prog"] = _build()
    return _CACHE["prog"]


def kernel(**inputs):
    nc = _get_program()
    in_maps = _prep_inputs(inputs)
    res = bass_utils.run_bass_kernel_spmd(nc, in_maps, core_ids=list(range(NCORES)))
    out = np.empty((T, B, 2 * H), np.float32)
    for c in range(NCORES):
        o = res.results[c]["out"].astype(np.float32)  # [T,2,128,(k,b)]
        o = o.reshape(T, 2, 128, 2, 32)
        o = o.transpose(0, 4, 1, 3, 2).reshape(T, BC, 512)
        out[:, c * BC:(c + 1) * BC, :] = o
    out *= 2.0
    return out
